# revision 1
# baseline (speedup 1.0000x reference)
"""Trainium2 Bass kernel for nn_BailingMoEAttention (B=2, S=2048, HID=2048,
NH=16, NKV=4, HD=128) on 8 NeuronCores.

Sharding: core c -> (batch b = c//4, kv-group g = c%4). Each core computes the
4 query heads sharing kv head g for batch b against its Wq/Wk/Wv column slices
and Wc row slice, producing a partial [S, HID] output; the host sums the 4
group partials per batch (tensor-parallel across heads per the hint, data-
parallel over batch for the remaining factor of 2).

Kernel design (per core):
  - all matmul operands bf16 (fast weight loads via FWL, split Ldweights the
    PE reorder window pulls ahead; halved HBM traffic), fp32 PSUM accumulate
  - phase 1 (QKV+RMSNorm+rope+transposes) software-pipelined: QKV matmuls for
    tile st run ahead of the norm/rope tail of st-1 so PE never waits on DVE;
    DVE reads QKV PSUM directly; squares/drains on the otherwise idle scalar
    engine; K-rope on GpSimd; q/k transposed via PE for the attention layout
  - phase 2 attention works on transposed score tiles ST[k,q] = K-chunk^T@Q^T
    so exp(ST) (scalar engine) directly yields transposed probabilities for
    the AV matmul; causal masking of diagonal tiles = one DVE multiply with
    one of 4 precomputed masks; softmax denominators ride along as a ones
    column in V; normalization is a per-partition scalar multiply
  - out-projection rows are interleaved between attention heads of the next
    q block, overlapping PE-heavy out-proj with exp-bound attention stretches
  - chunked DMAs (weights/xt split across queues, consts issued from ACT/Pool
    queues) so the first matmul starts within a few us
  - output bf16; host upcasts and sums the 4 partials per batch

reps>1 compiles a NEFF that runs the whole body (including weight reloads)
back-to-back reps times — used by test.py to measure steady-state body time
on hardware as (wall(reps=R)-wall(reps=1))/(R-1), which cancels the ~70ms
axon round-trip and ~0.6ms/call client dispatch overhead that would otherwise
swamp the measurement.
"""
import sys
sys.path.insert(0, "/opt/trn_rl_repo")

from contextlib import ExitStack

import numpy as np

import concourse.bass as bass
import concourse.tile as tile
from concourse import bacc, mybir
from concourse.masks import make_identity

F32 = mybir.dt.float32
BF16 = mybir.dt.bfloat16

B, S, HID = 2, 2048, 2048
NH, NKV, HD = 16, 4, 128
NHL = NH // NKV          # query heads per kv group (= per core)
DQ = NHL * HD
EPS = 1e-6
THETA = 10000.0
N_CORES = 8


def _build(reps=1, phases="123"):
    n_st = S // 128
    n_hc = HID // 128
    n_qb = S // 512
    half = HD // 2

    nc = bacc.Bacc("TRN2", target_bir_lowering=False, debug=False, num_devices=1)
    xt_d = nc.dram_tensor("xt", [HID, S], BF16, kind="ExternalInput").ap()
    wq_d = nc.dram_tensor("wq", [HID, DQ], BF16, kind="ExternalInput").ap()
    wkv_d = nc.dram_tensor("wkv", [HID, 2 * HD], BF16, kind="ExternalInput").ap()
    wc_d = nc.dram_tensor("wc", [DQ, HID], BF16, kind="ExternalInput").ap()
    qs_d = nc.dram_tensor("qs", [DQ], F32, kind="ExternalInput").ap()
    ks_d = nc.dram_tensor("ks", [HD], F32, kind="ExternalInput").ap()
    cos_d = nc.dram_tensor("cos", [S, half], BF16, kind="ExternalInput").ap()
    sin_d = nc.dram_tensor("sin", [S, half], BF16, kind="ExternalInput").ap()
    out_d = nc.dram_tensor("out", [S, HID], BF16, kind="ExternalOutput").ap()

    with tile.TileContext(nc) as tc, ExitStack() as ctx:
        const_p = ctx.enter_context(tc.tile_pool(name="const", bufs=1))
        big_p = ctx.enter_context(tc.tile_pool(name="big", bufs=1))

        ident = const_p.tile([128, 128], BF16)
        make_identity(nc, ident)
        eps_t = const_p.tile([128, 1], F32)
        nc.vector.memset(eps_t, EPS)
        # const loads issued from ACT/DVE queues so the SP queue leads with
        # the weight/xt chunks the first matmuls need
        qs_b = const_p.tile([128, DQ], F32)
        nc.scalar.dma_start(out=qs_b, in_=bass.AP(tensor=qs_d.tensor, offset=0,
                                                  ap=[[0, 128]] + qs_d.ap))
        ks_b = const_p.tile([128, HD], F32)
        nc.scalar.dma_start(out=ks_b, in_=bass.AP(tensor=ks_d.tensor, offset=0,
                                                  ap=[[0, 128]] + ks_d.ap))
        cs_sb = const_p.tile([128, n_st, half], BF16)
        sn_sb = const_p.tile([128, n_st, half], BF16)
        # 4 causal masks for diagonal score tiles: mask_j[p, x] = x - p - 128j >= 0
        cmask = const_p.tile([128, 4, 512], BF16)
        for j in range(4):
            nc.gpsimd.memset(cmask[:, j, :], 1.0)
            nc.gpsimd.affine_select(
                out=cmask[:, j, :], in_=cmask[:, j, :],
                compare_op=mybir.AluOpType.is_ge, fill=0.0,
                base=-128 * j, pattern=[[1, 512]], channel_multiplier=-1)
        nc.scalar.dma_start(out=cs_sb, in_=cos_d.rearrange("(t p) h -> p t h", p=128))
        nc.scalar.dma_start(out=sn_sb, in_=sin_d.rearrange("(t p) h -> p t h", p=128))

        qt_sb = big_p.tile([128, n_st, NHL, 128], BF16)  # [d,(st,head,qq)]
        kt_sb = big_p.tile([128, n_st, 128], BF16)       # [d,(chunk,kk)]
        v_sb = big_p.tile([128, n_st, HD + 1], BF16)     # [kk,(chunk, d|ones)]
        nc.vector.memset(v_sb[:, :, HD:HD + 1], 1.0)
        wq_sb = big_p.tile([128, n_hc, DQ], BF16)
        wkv_sb = big_p.tile([128, n_hc, 2 * HD], BF16)
        wc_sb = big_p.tile([128, NHL, HID], BF16)
        for _rep in range(reps):
            # chunked weight loads: first QKV matmul only waits for chunk 0,
            # and the pieces spread across DMA queues
            wq_r = wq_d.rearrange("(c p) n -> p c n", p=128)
            wkv_r = wkv_d.rearrange("(c p) n -> p c n", p=128)
            for s in ([slice(0, 1), slice(1, 2)] +
                      [slice(2 + 2 * i, 4 + 2 * i) for i in range(7)]):
                nc.sync.dma_start(out=wq_sb[:, s, :], in_=wq_r[:, s, :])
                nc.sync.dma_start(out=wkv_sb[:, s, :], in_=wkv_r[:, s, :])
            # wc isn't needed until the first out-proj (~half-way in); issue
            # from the idle Pool queue to keep SP free for xt streaming
            wc_r = wc_d.rearrange("(c p) n -> p c n", p=128)
            for c4 in range(4):
                s = slice(c4, c4 + 1)
                nc.gpsimd.dma_start(out=wc_sb[:, s, :], in_=wc_r[:, s, :])

            # ============ Phase 1: QKV + norm + rope + transposes ============
            with tc.tile_pool(name="p1xt", bufs=3) as xt_p, \
                 tc.tile_pool(name="p1q", bufs=2, space="PSUM") as qps_p, \
                 tc.tile_pool(name="p1kv", bufs=2, space="PSUM") as kvps_p, \
                 tc.tile_pool(name="p1tq", bufs=2, space="PSUM") as tq_p, \
                 tc.tile_pool(name="p1tk", bufs=2, space="PSUM") as tk_p, \
                 tc.tile_pool(name="p1tmp", bufs=3) as tmp_p:
                def p1_tail(st, q_ps, kv_ps):
                        # rms stats: squares on ACT (PSUM-friendly), reduce on DVE
                        sq = tmp_p.tile([128, DQ], BF16, tag="sq")
                        ssq = tmp_p.tile([128, NHL + 1], F32, tag="ssq")
                        nc.scalar.activation(sq, q_ps,
                                             mybir.ActivationFunctionType.Square)
                        nc.vector.tensor_reduce(
                            out=ssq[:, 0:NHL],
                            in_=sq.rearrange("p (h d) -> p h d", d=HD),
                            op=mybir.AluOpType.add, axis=mybir.AxisListType.X)
                        sqk = tmp_p.tile([128, HD], BF16, tag="sqk")
                        nc.scalar.activation(sqk, kv_ps[:, 0:HD],
                                             mybir.ActivationFunctionType.Square)
                        nc.vector.tensor_reduce(
                            out=ssq[:, NHL:NHL + 1], in_=sqk,
                            op=mybir.AluOpType.add, axis=mybir.AxisListType.X)
                        rstd = tmp_p.tile([128, NHL + 1], F32, tag="rstd")
                        nc.scalar.activation(rstd, ssq, mybir.ActivationFunctionType.Sqrt,
                                             bias=eps_t, scale=1.0 / HD)
                        nc.vector.reciprocal(rstd, rstd)
                        # combined scale = qs * rstd[head]  (broadcast along d)
                        comb = tmp_p.tile([128, DQ], F32, tag="comb")
                        rstd_b = bass.AP(
                            tensor=rstd.tensor, offset=rstd.offset,
                            ap=[rstd.ap[0], [rstd.ap[-1][0], NHL], [0, HD]])
                        nc.vector.tensor_mul(
                            comb.rearrange("p (h d) -> p h d", d=HD), qs_b.rearrange(
                                "p (h d) -> p h d", d=HD), rstd_b)
                        qn = tmp_p.tile([128, DQ], BF16, tag="qn")
                        nc.vector.tensor_mul(qn, q_ps, comb)
                        combk = tmp_p.tile([128, HD], F32, tag="combk")
                        nc.vector.tensor_scalar_mul(combk, ks_b, rstd[:, NHL:NHL + 1])
                        kn = tmp_p.tile([128, HD], BF16, tag="kn")
                        nc.vector.tensor_mul(kn, kv_ps[:, 0:HD], combk)
                        nc.scalar.activation(v_sb[:, st, 0:HD], kv_ps[:, HD:2 * HD],
                                             mybir.ActivationFunctionType.Copy)

                        # rope (all bf16, SBUF)
                        qr = tmp_p.tile([128, DQ], BF16, tag="qr")
                        kr = tmp_p.tile([128, HD], BF16, tag="kr")

                        def rope(dst, src, nh, eng=None):
                            eng = eng or nc.vector
                            s3 = src.rearrange("p (h two d) -> p h two d", h=nh, two=2)
                            d3 = dst.rearrange("p (h two d) -> p h two d", h=nh, two=2)
                            x1, x2 = s3[:, :, 0, :], s3[:, :, 1, :]
                            o1, o2 = d3[:, :, 0, :], d3[:, :, 1, :]
                            cs_t = cs_sb[:, st, :]
                            sn_t = sn_sb[:, st, :]
                            cb = bass.AP(tensor=cs_t.tensor, offset=cs_t.offset,
                                         ap=[cs_t.ap[0], [0, nh]] + cs_t.ap[1:])
                            sb = bass.AP(tensor=sn_t.tensor, offset=sn_t.offset,
                                         ap=[sn_t.ap[0], [0, nh]] + sn_t.ap[1:])
                            t1 = tmp_p.tile([128, nh, half], BF16, tag="ropet1")
                            t2 = tmp_p.tile([128, nh, half], BF16, tag="ropet2")
                            eng.tensor_mul(t1, x1, cb)
                            eng.tensor_mul(t2, x2, sb)
                            eng.tensor_sub(o1, t1, t2)
                            eng.tensor_mul(t1, x2, cb)
                            eng.tensor_mul(t2, x1, sb)
                            eng.tensor_add(o2, t1, t2)

                        rope(qr, qn, NHL)
                        rope(kr, kn, 1, eng=nc.gpsimd)
                        tq_ps = tq_p.tile([128, DQ], BF16, tag="tq")
                        for h in range(NHL):
                            nc.tensor.transpose(tq_ps[:, h * HD:(h + 1) * HD],
                                                qr[:, h * HD:(h + 1) * HD], ident)
                        tk_ps = tk_p.tile([128, HD], BF16, tag="tk")
                        nc.tensor.transpose(tk_ps[:], kr, ident)
                        # scalar engine is idle in phase 1 — put PSUM drains there
                        nc.scalar.activation(qt_sb[:, st, :, :], tq_ps,
                                             mybir.ActivationFunctionType.Copy)
                        nc.scalar.activation(kt_sb[:, st, :], tk_ps,
                                             mybir.ActivationFunctionType.Copy)

                # software pipeline: QKV matmuls for st run ahead of the
                # norm/rope/transpose tail of st-1, keeping PE fed while
                # DVE/ACT catch up
                pending = None
                for sg in range(n_st // 2):
                    xt_t = xt_p.tile([128, n_hc, 256], BF16)
                    xt_r = xt_d[:, sg * 256:(sg + 1) * 256].rearrange(
                        "(c p) s -> p c s", p=128)
                    first = [slice(0, 1), slice(1, 2), slice(2, 4)] \
                        if sg == 0 else [slice(0, 4)]
                    for s in first + [slice(4 * i, 4 * (i + 1)) for i in (1, 2, 3)]:
                        nc.scalar.dma_start(out=xt_t[:, s, :], in_=xt_r[:, s, :])
                    for t in range(2):
                        st = sg * 2 + t
                        q_ps = qps_p.tile([128, DQ], F32, tag="qps")
                        kv_ps = kvps_p.tile([128, 2 * HD], F32, tag="kvps")
                        for c in range(n_hc):
                            lhs = xt_t[:, c, t * 128:(t + 1) * 128]
                            nc.tensor.matmul(q_ps[:], lhs, wq_sb[:, c, :],
                                             start=(c == 0), stop=(c == n_hc - 1))
                            nc.tensor.matmul(kv_ps[:], lhs, wkv_sb[:, c, :],
                                             start=(c == 0), stop=(c == n_hc - 1))
                        if pending is not None:
                            p1_tail(*pending)
                        pending = (st, q_ps, kv_ps)
                p1_tail(*pending)

            if phases == "1":
                nc.sync.dma_start(
                    out=out_d[0:128, :],
                    in_=qt_sb.rearrange("p a b c -> p (a b c)")[:, 0:HID])
                continue
            # full normalized O^T, [d, (head, q)] bf16 (2 MB)
            ot_full = big_p.tile([128, NHL, S], BF16)

            # ===== Phase 2+3: attention with interleaved out-proj rows =====
            # Out-proj for q-row r (128 rows) is ready once q block r//4 finished
            # attention; we run one out-proj row after each head of the NEXT q
            # block so PE-heavy out-proj overlaps the ACT-heavy (exp) attention.
            with tc.tile_pool(name="a_st", bufs=2, space="PSUM") as st_ps_p, \
                 tc.tile_pool(name="a_o", bufs=2, space="PSUM") as o_ps_p, \
                 tc.tile_pool(name="a_ot", bufs=1, space="PSUM") as ot_ps_p, \
                 tc.tile_pool(name="a_op", bufs=3, space="PSUM") as op_ps_p, \
                 tc.tile_pool(name="a_pt", bufs=17) as pt_p, \
                 tc.tile_pool(name="a_sb", bufs=2) as at_sb_p, \
                 tc.tile_pool(name="a_r", bufs=8) as r_p, \
                 tc.tile_pool(name="a_out", bufs=3) as out_p:

                def attn_head(qb, h):
                    nkc = 4 * (qb + 1)
                    qt_rhs = qt_sb[:, qb * 4:(qb + 1) * 4, h, :]
                    # scores + exp for the whole k band, kept in SBUF
                    ptus = []
                    for kc in range(nkc):
                        st_ps = st_ps_p.tile([128, 512], F32, tag="st")
                        nc.tensor.matmul(st_ps[:], kt_sb[:, kc, :], qt_rhs,
                                         start=True, stop=True)
                        ptu = pt_p.tile([128, 512], BF16, tag="ptu")
                        nc.scalar.activation(ptu, st_ps,
                                             mybir.ActivationFunctionType.Exp)
                        if kc >= 4 * qb:
                            j = kc - 4 * qb
                            nc.vector.tensor_mul(ptu, ptu, cmask[:, j, :])
                        ptus.append(ptu)
                    # AV in two halves (2 PSUM accumulator banks)
                    o_sb = at_sb_p.tile([128, 4, HD], BF16, tag="o_sb")
                    for half in range(2):
                        o_ps = [o_ps_p.tile([128, HD + 1], F32, tag="o",
                                            name=f"ops{_t}") for _t in range(2)]
                        for kc in range(nkc):
                            for t2 in range(2):
                                t = half * 2 + t2
                                nc.tensor.matmul(
                                    o_ps[t2][:],
                                    ptus[kc][:, t * 128:(t + 1) * 128],
                                    v_sb[:, kc, :],
                                    start=(kc == 0), stop=(kc == nkc - 1))
                        for t2 in range(2):
                            t = half * 2 + t2
                            op = o_ps[t2][:]
                            r_t = r_p.tile([128, 1], F32, tag="r_t")
                            nc.vector.reciprocal(r_t, op[:, HD:HD + 1])
                            nc.vector.tensor_scalar_mul(o_sb[:, t, :], op[:, 0:HD],
                                                        r_t)
                    return o_sb

                def attn_head_fin(qb, h, o_sb):
                    # deferred past the interleaved out-proj so the PE isn't
                    # FIFO-blocked waiting on the DVE normalize
                    ot_ps = ot_ps_p.tile([128, 512], BF16, tag="ot")
                    for t in range(4):
                        nc.tensor.transpose(ot_ps[:, t * 128:(t + 1) * 128],
                                            o_sb[:, t, :], ident)
                    nc.vector.tensor_copy(ot_full[:, h, qb * 512:(qb + 1) * 512],
                                          ot_ps)

                def outproj_row(r):
                    o_row = out_p.tile([128, 4, 512], BF16, tag="o_row")
                    for hs in range(4):
                        op_ps = op_ps_p.tile([128, 512], F32, tag="op")
                        for h in range(NHL):
                            nc.tensor.matmul(
                                op_ps[:],
                                ot_full[:, h, r * 128:(r + 1) * 128],
                                wc_sb[:, h, hs * 512:(hs + 1) * 512],
                                start=(h == 0), stop=(h == NHL - 1))
                        if hs % 2 == 0:
                            nc.scalar.activation(o_row[:, hs, :], op_ps,
                                                 mybir.ActivationFunctionType.Copy)
                        else:
                            nc.vector.tensor_copy(o_row[:, hs, :], op_ps)
                    nc.sync.dma_start(
                        out=out_d[r * 128:(r + 1) * 128, :], in_=o_row)

                if phases == "12":
                    for qb in range(n_qb):
                        for h in range(NHL):
                            o_sb = attn_head(qb, h)
                            attn_head_fin(qb, h, o_sb)
                    nc.sync.dma_start(
                        out=out_d[0:128, :],
                        in_=ot_full.rearrange("p a b -> p (a b)")[:, 0:HID])
                else:
                    for qb in range(n_qb):
                        for h in range(NHL):
                            o_sb = attn_head(qb, h)
                            if qb >= 1:
                                outproj_row(4 * (qb - 1) + h)
                            attn_head_fin(qb, h, o_sb)
                    for r in range(4 * (n_qb - 1), 4 * n_qb):
                        outproj_row(r)
    nc.compile()
    return nc


def _rope_tables(positions_1d):
    half = HD // 2
    inv_freq = 1.0 / (THETA ** (np.arange(half, dtype=np.float64) / half))
    ang = positions_1d.astype(np.float64)[:, None] * inv_freq[None, :]
    return np.cos(ang), np.sin(ang)


def _prep_shared(hidden_states, positions, Wq, Wk, Wv, Wc, q_scale, k_scale):
    """Per-batch and per-group host tensors, shared across cores."""
    import ml_dtypes
    bf16 = ml_dtypes.bfloat16
    c = float(HD) ** -0.25
    xt = [np.ascontiguousarray(hidden_states[b].T).astype(bf16) for b in range(B)]
    tabs = []
    for b in range(B):
        cos, sin = _rope_tables(np.asarray(positions[b]))
        tabs.append((cos.astype(bf16), sin.astype(bf16)))
    wq = [np.ascontiguousarray(Wq[:, g * DQ:(g + 1) * DQ]).astype(bf16)
          for g in range(NKV)]
    wkv = [np.ascontiguousarray(
        np.concatenate([Wk[:, g * HD:(g + 1) * HD],
                        Wv[:, g * HD:(g + 1) * HD]], axis=1)).astype(bf16)
        for g in range(NKV)]
    wc = [np.ascontiguousarray(Wc[g * DQ:(g + 1) * DQ, :]).astype(bf16)
          for g in range(NKV)]
    qs = np.tile(np.asarray(q_scale, np.float32) * c, NHL)
    ks = np.asarray(k_scale, np.float32) * c
    return xt, tabs, wq, wkv, wc, qs, ks


def _core_inputs_all(inputs):
    xt, tabs, wq, wkv, wc, qs, ks = _prep_shared(**inputs)
    in_maps = []
    for core in range(N_CORES):
        b, g = divmod(core, NKV)
        in_maps.append({
            "xt": xt[b], "wq": wq[g], "wkv": wkv[g], "wc": wc[g],
            "qs": qs, "ks": ks, "cos": tabs[b][0], "sin": tabs[b][1],
        })
    return in_maps


_CACHED = {}


def kernel(hidden_states, positions, Wq, Wk, Wv, Wc, q_scale, k_scale):
    from concourse import bass_utils

    inputs = dict(hidden_states=np.asarray(hidden_states, np.float32),
                  positions=np.asarray(positions),
                  Wq=np.asarray(Wq, np.float32), Wk=np.asarray(Wk, np.float32),
                  Wv=np.asarray(Wv, np.float32), Wc=np.asarray(Wc, np.float32),
                  q_scale=np.asarray(q_scale, np.float32),
                  k_scale=np.asarray(k_scale, np.float32))

    if "nc" not in _CACHED:
        _CACHED["nc"] = _build()
    nc = _CACHED["nc"]

    in_maps = _core_inputs_all(inputs)
    res = bass_utils.run_bass_kernel_spmd(nc, in_maps, core_ids=list(range(N_CORES)))
    out = np.zeros((B, S, HID), np.float32)
    for core in range(N_CORES):
        b, _ = divmod(core, NKV)
        out[b] += np.asarray(res.results[core]["out"], np.float32)
    return out

